# revision 24
# baseline (speedup 1.0000x reference)
"""AttnPooling Trainium2 kernel: 8-core data-parallel, transposed-token layout.

Per item (of NI=16 per core): x is (D=128, K=4096) fp32 in HBM, host-packed to
bf16 "Xt" layout: SBUF tile (128 part, 32 blocks x [128 x-cols | 1 ones | 1 pad])
where element [p, t*130+d] = x[d, t*128+p].  Token k = t*128+p lives as a
length-128 d-row segment on partition p.

  mean_raw^T (1,129) = sum_t  mfold[:,t]^T @ XT[:, t-block]      (PE, k-contract)
                       col 128 = c (mask count, from the ones col)
  vT (1,128) = ((mean_col^T @ CQK) * (1/c)) + w0^T               (PE + DVE STT)
  VB (128,128) = ones  (x)  vT                                   (PE broadcast)
  Q = XT (.) VB-bcast  -> 7-level binary tree sum over d         (DVE, fp16)
  s_fold (128,32);  E = exp(s*SD);  P = mfold (.) E              (ACT + DVE)
  pooled^T|Z (1,129) = sum_t P[:,t]^T @ XT[:, t-block]           (PE, k-contract)
  out row = pooled * (1/Z)                                       (ACT copy+scale)

All heavy reductions run on PE (partition contraction) or the DVE at 2x bf16;
no 1x-rate custom-DVE pass and no on-chip mask/e broadcast materialization.
"""

import sys

sys.path.insert(0, "/opt/trn_rl_repo")

import numpy as np
from contextlib import ExitStack

NI = 16  # items per core
D = 128
K = 4096
T = 32  # k-tiles per item
BL = 130  # padded block width: 128 x-cols + ones col + pad col
QW = 136  # padded product-block width (keeps pool view non-coalescible)
NCORES = 8
SD = 1.0 / np.sqrt(128.0)

_CACHE = {}


def _build():
    import concourse.bass as bass
    import concourse.tile as tile
    from concourse import bacc, mybir

    dt = mybir.dt
    Alu = mybir.AluOpType
    Act = mybir.ActivationFunctionType

    nc = bacc.Bacc(
        "TRN2", target_bir_lowering=False, debug=False, num_devices=NCORES
    )
    x_d = nc.dram_tensor("x", [NI, D, T * BL], dt.bfloat16, kind="ExternalInput").ap()
    mf_d = nc.dram_tensor("mf", [D, NI * T * 2], dt.bfloat16, kind="ExternalInput").ap()
    wq_d = nc.dram_tensor("Wq", [D, D], dt.float32, kind="ExternalInput").ap()
    wk_d = nc.dram_tensor("Wk", [D, D], dt.float32, kind="ExternalInput").ap()
    bq_d = nc.dram_tensor("bq", [D, 1], dt.float32, kind="ExternalInput").ap()
    out_d = nc.dram_tensor("out", [1, NI * D], dt.float32, kind="ExternalOutput").ap()

    with tile.TileContext(nc) as tc, ExitStack() as ctx:
        # SBUF pools
        xp = ctx.enter_context(tc.tile_pool(name="xp", bufs=5))
        qp = ctx.enter_context(tc.tile_pool(name="qp", bufs=2))
        rp = ctx.enter_context(tc.tile_pool(name="rp", bufs=2))
        vp = ctx.enter_context(tc.tile_pool(name="vp", bufs=3))
        per = ctx.enter_context(tc.tile_pool(name="per", bufs=1))
        # PSUM pools: exactly 8 banks total
        meanp = ctx.enter_context(tc.tile_pool(name="meanp", bufs=2, space="PSUM"))
        poolp = ctx.enter_context(tc.tile_pool(name="poolp", bufs=2, space="PSUM"))
        chainp = ctx.enter_context(tc.tile_pool(name="chainp", bufs=2, space="PSUM"))
        vbp = ctx.enter_context(tc.tile_pool(name="vbp", bufs=2, space="PSUM"))

        # persistent tiles
        wq = per.tile([D, D], dt.float32, tag="wq")
        wk = per.tile([D, D], dt.float32, tag="wk")
        bq = per.tile([D, 1], dt.float32, tag="bq")
        # MF/P use stride-2 columns so every (128,1) LDWEIGHTS slice is 4B-aligned
        MF = per.tile([D, NI * T * 2], dt.bfloat16, tag="MF")
        cqk = per.tile([D, D], dt.bfloat16, tag="cqk")
        w0T = per.tile([1, D], dt.float32, tag="w0T")
        ones1 = per.tile([1, D], dt.bfloat16, tag="ones1")
        onebb = per.tile([1, 1], dt.bfloat16, tag="onebb")
        R7 = per.tile([D, NI * T], dt.float32, tag="R7")
        E = per.tile([D, NI * T], dt.bfloat16, tag="E")
        P = per.tile([D, NI * T * 2], dt.bfloat16, tag="P")
        cinv = per.tile([1, NI], dt.float32, tag="cinv")
        zinv = per.tile([1, NI], dt.float32, tag="zinv")
        outt = per.tile([1, NI * D], dt.float32, tag="outt")

        # ---- setup ----
        nc.sync.dma_start(wq[:, :], wq_d[:, :])
        nc.sync.dma_start(wk[:, :], wk_d[:, :])
        nc.sync.dma_start(bq[:, :], bq_d[:, :])
        nc.sync.dma_start(MF[:, :], mf_d[:, :])
        nc.vector.memset(ones1[:, :], 1.0)
        nc.vector.memset(onebb[:, :], 1.0)

        cqk_ps = vbp.tile([D, D], dt.float32, tag="vb", name="cqk_ps")
        nc.tensor.matmul(cqk_ps[:, :], wq[:, :], wk[:, :], start=True, stop=True)
        nc.scalar.copy(cqk[:, :], cqk_ps[:, :])
        w0_ps = vbp.tile([1, D], dt.float32, tag="vb", name="w0_ps")
        nc.tensor.matmul(w0_ps[:, :], bq[:, :], wk[:, :], start=True, stop=True)
        nc.scalar.copy(w0T[:, :], w0_ps[:, :])

        # per-item 1/c at setup: c = sum over (p,t) of the mask fold
        one128f = per.tile([D, 1], dt.float32, tag="one128f")
        cpart = per.tile([D, NI], dt.float32, tag="cpart")
        nc.vector.memset(one128f[:, :], 1.0)
        mf3s = MF[:, :].rearrange("p (i t k) -> p i (t k)", i=NI, k=2)
        nc.vector.tensor_reduce(
            cpart[:, :], mf3s, axis=mybir.AxisListType.X, op=Alu.add
        )
        crow_ps = chainp.tile([1, NI], dt.float32, tag="ch", name="crow_ps")
        nc.tensor.matmul(
            crow_ps[:, :], one128f[:, :], cpart[:, :], start=True, stop=True
        )
        nc.vector.reciprocal(cinv[:, :], crow_ps[:, :])

        xts = [None] * NI
        vbs = [None] * NI

        def load_phase(i):
            xt = xp.tile([D, T * BL], dt.bfloat16, tag="x", name=f"x_{i}")
            nc.sync.dma_start(xt[:, :], x_d[i, :, :])
            xts[i] = xt

        meanTs = [None] * NI

        def mean_mms(i):
            xt = xts[i]
            meanps = meanp.tile([1, D], dt.float32, tag="m", name=f"mps_{i}")
            for t in range(T):
                nc.tensor.matmul(
                    meanps[:, :],
                    MF[:, 2 * (i * T + t) : 2 * (i * T + t) + 1],
                    xt[:, t * BL : t * BL + D],
                    start=(t == 0),
                    stop=(t == T - 1),
                )
            meanT = vp.tile([1, D], dt.bfloat16, tag="mT", name=f"mT_{i}")
            nc.scalar.copy(meanT[:, :], meanps[:, :])
            meanTs[i] = meanT

        def chain_phase(i):
            meanT = meanTs[i]
            mcps = chainp.tile([D, 1], dt.float32, tag="ch", name=f"mc_{i}")
            nc.tensor.matmul(
                mcps[:, :], meanT[:, :], onebb[:, :], start=True, stop=True
            )
            mcol = vp.tile([D, 1], dt.bfloat16, tag="mc", name=f"mcol_{i}")
            nc.scalar.copy(mcol[:, :], mcps[:, :])
            vTps = chainp.tile([1, D], dt.float32, tag="ch", name=f"vT_{i}")
            nc.tensor.matmul(vTps[:, :], mcol[:, :], cqk[:, :], start=True, stop=True)
            vTsb = vp.tile([1, D], dt.bfloat16, tag="vTs", name=f"vTs_{i}")
            nc.vector.scalar_tensor_tensor(
                vTsb[:, :],
                vTps[:, :],
                cinv[0:1, i : i + 1],
                w0T[:, :],
                op0=Alu.mult,
                op1=Alu.add,
            )
            vbps = vbp.tile([D, D], dt.float32, tag="vb", name=f"vbp_{i}")
            nc.tensor.matmul(vbps[:, :], ones1[:, :], vTsb[:, :], start=True, stop=True)
            vb = vp.tile([D, D], dt.bfloat16, tag="vbs", name=f"vb_{i}")
            nc.scalar.copy(vb[:, :], vbps[:, :])
            vbs[i] = vb

        def attn_dve(i):
            xt, vb = xts[i], vbs[i]
            x3 = xt[:, :].rearrange("p (t e) -> p t e", e=BL)[:, :, 0:D]
            v3 = vb[:, :].unsqueeze(1).broadcast_to((D, T, D))
            q = qp.tile([D, K], dt.bfloat16, tag="q", name=f"q_{i}")
            nc.vector.tensor_tensor(
                q[:, :].rearrange("p (t d) -> p t d", d=D), x3, v3, op=Alu.mult
            )
            cur, w = q, D
            for lv in range(6):
                w //= 2
                # last level fp32 so the final pair-add operands stay 4B-aligned
                rdt = dt.float32 if lv == 5 else dt.float16
                r = rp.tile([D, T * w], rdt, tag=f"r{lv}", name=f"r{lv}_{i}")
                c3 = cur[:, :].rearrange("p (t d) -> p t d", d=2 * w)
                # tree tail on the otherwise-idle GpSimd engine
                eng = nc.vector if lv < 3 else nc.gpsimd
                eng.tensor_tensor(
                    r[:, :].rearrange("p (t d) -> p t d", d=w),
                    c3[:, :, 0:w],
                    c3[:, :, w : 2 * w],
                    op=Alu.add,
                )
                cur = r
            c3 = cur[:, :].rearrange("p (t d) -> p t d", d=2)
            nc.gpsimd.tensor_tensor(
                R7[:, i * T : (i + 1) * T].unsqueeze(2),
                c3[:, :, 0:1],
                c3[:, :, 1:2],
                op=Alu.add,
            )
            nc.scalar.activation(
                E[:, i * T : (i + 1) * T],
                R7[:, i * T : (i + 1) * T],
                Act.Exp,
                scale=SD,
            )
            mf3 = MF[:, :].rearrange("p (j k) -> p j k", k=2)
            p3 = P[:, :].rearrange("p (j k) -> p j k", k=2)
            nc.vector.tensor_tensor(
                p3[:, i * T : (i + 1) * T, 0:1],
                E[:, i * T : (i + 1) * T].unsqueeze(2),
                mf3[:, i * T : (i + 1) * T, 0:1],
                op=Alu.mult,
            )

        def attn_pe(i):
            xt = xts[i]
            pps = poolp.tile([1, D + 1], dt.float32, tag="p", name=f"pps_{i}")
            for t in range(T):
                nc.tensor.matmul(
                    pps[:, :],
                    P[:, 2 * (i * T + t) : 2 * (i * T + t) + 1],
                    xt[:, t * BL : t * BL + D + 1],
                    start=(t == 0),
                    stop=(t == T - 1),
                )
            # single PSUM reader: scalar copies [pooled | Z] to SBUF first
            po129 = vp.tile([1, D + 1], dt.float32, tag="po", name=f"po_{i}")
            nc.scalar.copy(po129[:, :], pps[:, :])
            nc.vector.reciprocal(zinv[0:1, i : i + 1], po129[0:1, D : D + 1])
            nc.scalar.activation(
                outt[0:1, i * D : (i + 1) * D],
                po129[0:1, 0:D],
                Act.Copy,
                scale=zinv[0:1, i : i + 1],
            )

        # software pipeline, one iteration = one steady-state cycle:
        #   load(i+3) | mean+v-chain(i+1) | scores/tree DVE(i) | pooled PE(i-1)
        # the v-chain for item i completes a full cycle before attn_dve(i)
        # needs VB(i), so the DVE never stalls on the chain's serial latency.
        load_phase(0)
        load_phase(1)
        load_phase(2)
        mean_mms(0)
        chain_phase(0)
        for i in range(NI):
            if i + 3 < NI:
                load_phase(i + 3)
            if i + 1 < NI:
                mean_mms(i + 1)
            if i >= 1:
                attn_pe(i - 1)
            if i + 1 < NI:
                chain_phase(i + 1)
            attn_dve(i)
        attn_pe(NI - 1)

        nc.sync.dma_start(out_d[:, :], outt[:, :])

    nc.compile()
    return nc


def _get_nc():
    if "nc" not in _CACHE:
        _CACHE["nc"] = _build()
    return _CACHE["nc"]


def _pack_inputs(x, mask):
    """Host-side layout prep: bf16 Xt-fold with ones/pad columns, mask fold."""
    import ml_dtypes

    bf16 = ml_dtypes.bfloat16
    B, N, d, H, W = x.shape
    M = B * N  # 128 items
    xr = np.asarray(x, dtype=np.float32).reshape(M, d, T, d)  # [item, d, t, p]
    xt = np.transpose(xr, (0, 3, 2, 1))  # [item, p, t, d]
    xtp = np.zeros((M, d, T, BL), dtype=bf16)
    xtp[:, :, :, 0:d] = xt.astype(bf16)
    xtp[:, :, :, d] = np.asarray(1.0, dtype=bf16)
    xtp = xtp.reshape(M, d, T * BL)

    mr = np.asarray(mask, dtype=np.float32).reshape(M, T, d)  # [item, t, p]
    mfo = np.transpose(mr, (0, 2, 1)).astype(bf16)  # [item, p, t]
    return xtp, mfo


def _make_in_maps(inputs):
    x, mask = inputs["x"], inputs["mask"]
    Wq, bq, Wk = inputs["Wq"], inputs["bq"], inputs["Wk"]
    xtp, mfo = _pack_inputs(x, mask)
    wqc = np.ascontiguousarray(Wq.astype(np.float32))
    wkc = np.ascontiguousarray(Wk.astype(np.float32))
    bq2 = np.ascontiguousarray(bq.reshape(D, 1).astype(np.float32))
    in_maps = []
    for c in range(NCORES):
        s = slice(c * NI, (c + 1) * NI)
        mfc = np.zeros((D, NI * T, 2), dtype=mfo.dtype)
        mfc[:, :, 0] = np.transpose(mfo[s], (1, 0, 2)).reshape(D, NI * T)
        mfc = np.ascontiguousarray(mfc.reshape(D, NI * T * 2))
        in_maps.append(
            {
                "x": np.ascontiguousarray(xtp[s]),
                "mf": mfc,
                "Wq": wqc,
                "Wk": wkc,
                "bq": bq2,
            }
        )
    return in_maps


def _gather(res, inputs):
    B, N, d = inputs["x"].shape[:3]
    parts = [
        np.asarray(res.results[c]["out"], dtype=np.float32).reshape(NI, d)
        for c in range(NCORES)
    ]
    return np.concatenate(parts, axis=0).reshape(B, N, d)


def kernel(x, mask, Wq, bq, Wk, bk):
    from concourse.bass_utils import run_bass_kernel_spmd

    inputs = {"x": x, "mask": mask, "Wq": Wq, "bq": bq, "Wk": Wk, "bk": bk}
    nc = _get_nc()
    in_maps = _make_in_maps(inputs)
    res = run_bass_kernel_spmd(nc, in_maps, core_ids=list(range(NCORES)))
    return _gather(res, inputs)


# revision 29
# speedup vs baseline: 1.2177x; 1.2177x over previous
"""AttnPooling Trainium2 kernel: 8-core data-parallel, transposed-token layout.

Per item (of NI=16 per core): x is (D=128, K=4096) fp32 in HBM, host-packed to
bf16 "Xt" layout: SBUF tile (128 part, 32 blocks x [128 x-cols | 1 ones | 1 pad])
where element [p, t*130+d] = x[d, t*128+p].  Token k = t*128+p lives as a
length-128 d-row segment on partition p.

  mean_raw^T (1,129) = sum_t  mfold[:,t]^T @ XT[:, t-block]      (PE, k-contract)
                       col 128 = c (mask count, from the ones col)
  vT (1,128) = ((mean_col^T @ CQK) * (1/c)) + w0^T               (PE + DVE STT)
  VB (128,128) = ones  (x)  vT                                   (PE broadcast)
  Q = XT (.) VB-bcast  -> 7-level binary tree sum over d         (DVE, fp16)
  s_fold (128,32);  E = exp(s*SD);  P = mfold (.) E              (ACT + DVE)
  pooled^T|Z (1,129) = sum_t P[:,t]^T @ XT[:, t-block]           (PE, k-contract)
  out row = pooled * (1/Z)                                       (ACT copy+scale)

All heavy reductions run on PE (partition contraction) or the DVE at 2x bf16;
no 1x-rate custom-DVE pass and no on-chip mask/e broadcast materialization.
"""

import sys

sys.path.insert(0, "/opt/trn_rl_repo")

import numpy as np
from contextlib import ExitStack

NI = 16  # items per core
D = 128
K = 4096
T = 32  # k-tiles per item
BL = 130  # padded block width: 128 x-cols + ones col + pad col
QW = 136  # padded product-block width (keeps pool view non-coalescible)
NCORES = 8
SD = 1.0 / np.sqrt(128.0)

_CACHE = {}


def _build():
    import concourse.bass as bass
    import concourse.tile as tile
    from concourse import bacc, mybir

    dt = mybir.dt
    Alu = mybir.AluOpType
    Act = mybir.ActivationFunctionType

    nc = bacc.Bacc(
        "TRN2", target_bir_lowering=False, debug=False, num_devices=NCORES
    )
    x_d = nc.dram_tensor("x", [NI, D, T * BL], dt.bfloat16, kind="ExternalInput").ap()
    mf_d = nc.dram_tensor("mf", [D, NI * T * 2], dt.bfloat16, kind="ExternalInput").ap()
    wq_d = nc.dram_tensor("Wq", [D, D], dt.float32, kind="ExternalInput").ap()
    wk_d = nc.dram_tensor("Wk", [D, D], dt.float32, kind="ExternalInput").ap()
    bq_d = nc.dram_tensor("bq", [D, 1], dt.float32, kind="ExternalInput").ap()
    out_d = nc.dram_tensor("out", [1, NI * D], dt.float32, kind="ExternalOutput").ap()

    with tile.TileContext(nc) as tc, ExitStack() as ctx:
        # SBUF pools
        xp = ctx.enter_context(tc.tile_pool(name="xp", bufs=6))
        qp = ctx.enter_context(tc.tile_pool(name="qp", bufs=2))
        rp = ctx.enter_context(tc.tile_pool(name="rp", bufs=2))
        vp = ctx.enter_context(tc.tile_pool(name="vp", bufs=3))
        per = ctx.enter_context(tc.tile_pool(name="per", bufs=1))
        # PSUM pools: exactly 8 banks total
        meanp = ctx.enter_context(tc.tile_pool(name="meanp", bufs=2, space="PSUM"))
        poolp = ctx.enter_context(tc.tile_pool(name="poolp", bufs=2, space="PSUM"))
        chainp = ctx.enter_context(tc.tile_pool(name="chainp", bufs=2, space="PSUM"))
        vbp = ctx.enter_context(tc.tile_pool(name="vbp", bufs=2, space="PSUM"))

        # persistent tiles
        wq = per.tile([D, D], dt.float32, tag="wq")
        wk = per.tile([D, D], dt.float32, tag="wk")
        bq = per.tile([D, 1], dt.float32, tag="bq")
        # MF/P use stride-2 columns so every (128,1) LDWEIGHTS slice is 4B-aligned
        MF = per.tile([D, NI * T * 2], dt.bfloat16, tag="MF")
        cqk = per.tile([D, D], dt.bfloat16, tag="cqk")
        w0T = per.tile([1, D], dt.float32, tag="w0T")
        ones1 = per.tile([1, D], dt.bfloat16, tag="ones1")
        onebb = per.tile([1, 1], dt.bfloat16, tag="onebb")
        R7 = per.tile([D, NI * T], dt.float32, tag="R7")
        E = per.tile([D, NI * T], dt.bfloat16, tag="E")
        P = per.tile([D, NI * T * 2], dt.bfloat16, tag="P")
        cinv = per.tile([1, NI], dt.float32, tag="cinv")
        zinv = per.tile([1, NI], dt.float32, tag="zinv")
        outt = per.tile([1, NI * D], dt.float32, tag="outt")

        # ---- setup ----
        nc.sync.dma_start(wq[:, :], wq_d[:, :])
        nc.sync.dma_start(wk[:, :], wk_d[:, :])
        nc.sync.dma_start(bq[:, :], bq_d[:, :])
        nc.sync.dma_start(MF[:, :], mf_d[:, :])
        nc.vector.memset(ones1[:, :], 1.0)
        nc.vector.memset(onebb[:, :], 1.0)

        cqk_ps = vbp.tile([D, D], dt.float32, tag="vb", name="cqk_ps")
        nc.tensor.matmul(cqk_ps[:, :], wq[:, :], wk[:, :], start=True, stop=True)
        nc.scalar.copy(cqk[:, :], cqk_ps[:, :])
        w0_ps = vbp.tile([1, D], dt.float32, tag="vb", name="w0_ps")
        nc.tensor.matmul(w0_ps[:, :], bq[:, :], wk[:, :], start=True, stop=True)
        nc.scalar.copy(w0T[:, :], w0_ps[:, :])

        # per-item 1/c at setup: c = sum over (p,t) of the mask fold
        one128f = per.tile([D, 1], dt.float32, tag="one128f")
        cpart = per.tile([D, NI], dt.float32, tag="cpart")
        nc.vector.memset(one128f[:, :], 1.0)
        mf3s = MF[:, :].rearrange("p (i t k) -> p i (t k)", i=NI, k=2)
        nc.vector.tensor_reduce(
            cpart[:, :], mf3s, axis=mybir.AxisListType.X, op=Alu.add
        )
        crow_ps = chainp.tile([1, NI], dt.float32, tag="ch", name="crow_ps")
        nc.tensor.matmul(
            crow_ps[:, :], one128f[:, :], cpart[:, :], start=True, stop=True
        )
        nc.vector.reciprocal(cinv[:, :], crow_ps[:, :])

        xts = [None] * NI
        vbs = [None] * NI

        def load_phase(i):
            xt = xp.tile([D, T * BL], dt.bfloat16, tag="x", name=f"x_{i}")
            nc.sync.dma_start(xt[:, :], x_d[i, :, :])
            xts[i] = xt

        meanTs = [None] * NI

        def mean_mms(i):
            xt = xts[i]
            meanps = meanp.tile([1, D], dt.float32, tag="m", name=f"mps_{i}")
            for t in range(T):
                nc.tensor.matmul(
                    meanps[:, :],
                    MF[:, 2 * (i * T + t) : 2 * (i * T + t) + 1],
                    xt[:, t * BL : t * BL + D],
                    start=(t == 0),
                    stop=(t == T - 1),
                )
            meanT = vp.tile([1, D], dt.bfloat16, tag="mT", name=f"mT_{i}")
            nc.scalar.copy(meanT[:, :], meanps[:, :])
            meanTs[i] = meanT

        mcols = [None] * NI
        vTpss = [None] * NI
        vTsbs = [None] * NI

        def chain_a(i):
            meanT = meanTs[i]
            mcps = chainp.tile([D, 1], dt.float32, tag="ch", name=f"mc_{i}")
            nc.tensor.matmul(
                mcps[:, :], meanT[:, :], onebb[:, :], start=True, stop=True
            )
            mcol = vp.tile([D, 1], dt.bfloat16, tag="mc", name=f"mcol_{i}")
            nc.scalar.copy(mcol[:, :], mcps[:, :])
            mcols[i] = mcol

        def chain_vT(i):
            vTps = chainp.tile([1, D], dt.float32, tag="ch", name=f"vT_{i}")
            nc.tensor.matmul(
                vTps[:, :], mcols[i][:, :], cqk[:, :], start=True, stop=True
            )
            vTpss[i] = vTps

        def chain_stt(i):
            vTsb = vp.tile([1, D], dt.bfloat16, tag="vTs", name=f"vTs_{i}")
            nc.vector.scalar_tensor_tensor(
                vTsb[:, :],
                vTpss[i][:, :],
                cinv[0:1, i : i + 1],
                w0T[:, :],
                op0=Alu.mult,
                op1=Alu.add,
            )
            vTsbs[i] = vTsb

        def chain_vb(i):
            vbps = vbp.tile([D, D], dt.float32, tag="vb", name=f"vbp_{i}")
            nc.tensor.matmul(
                vbps[:, :], ones1[:, :], vTsbs[i][:, :], start=True, stop=True
            )
            vb = vp.tile([D, D], dt.bfloat16, tag="vbs", name=f"vb_{i}")
            nc.scalar.copy(vb[:, :], vbps[:, :])
            vbs[i] = vb

        curs = [None] * NI

        def attn_dve(i):
            xt, vb = xts[i], vbs[i]
            x3 = xt[:, :].rearrange("p (t e) -> p t e", e=BL)[:, :, 0:D]
            v3 = vb[:, :].unsqueeze(1).broadcast_to((D, T, D))
            q = qp.tile([D, K], dt.bfloat16, tag="q", name=f"q_{i}")
            nc.vector.tensor_tensor(
                q[:, :].rearrange("p (t d) -> p t d", d=D), x3, v3, op=Alu.mult
            )
            cur, w = q, D
            for lv in range(3):
                w //= 2
                r = rp.tile([D, T * w], dt.float16, tag=f"r{lv}", name=f"r{lv}_{i}")
                c3 = cur[:, :].rearrange("p (t d) -> p t d", d=2 * w)
                nc.vector.tensor_tensor(
                    r[:, :].rearrange("p (t d) -> p t d", d=w),
                    c3[:, :, 0:w],
                    c3[:, :, w : 2 * w],
                    op=Alu.add,
                )
                cur = r
            curs[i] = (cur, w)

        def dve_tail(i):
            cur, w = curs[i]
            for lv in range(3, 6):
                w //= 2
                # last level fp32 so the final pair-add operands stay 4B-aligned
                rdt = dt.float32 if lv == 5 else dt.float16
                r = rp.tile([D, T * w], rdt, tag=f"r{lv}", name=f"r{lv}_{i}")
                c3 = cur[:, :].rearrange("p (t d) -> p t d", d=2 * w)
                nc.vector.tensor_tensor(
                    r[:, :].rearrange("p (t d) -> p t d", d=w),
                    c3[:, :, 0:w],
                    c3[:, :, w : 2 * w],
                    op=Alu.add,
                )
                cur = r
            c3 = cur[:, :].rearrange("p (t d) -> p t d", d=2)
            nc.vector.tensor_tensor(
                R7[:, i * T : (i + 1) * T].unsqueeze(2),
                c3[:, :, 0:1],
                c3[:, :, 1:2],
                op=Alu.add,
            )
            nc.scalar.activation(
                E[:, i * T : (i + 1) * T],
                R7[:, i * T : (i + 1) * T],
                Act.Exp,
                scale=SD,
            )

        def pmult_phase(i):
            mf3 = MF[:, :].rearrange("p (j k) -> p j k", k=2)
            p3 = P[:, :].rearrange("p (j k) -> p j k", k=2)
            nc.vector.tensor_tensor(
                p3[:, i * T : (i + 1) * T, 0:1],
                E[:, i * T : (i + 1) * T].unsqueeze(2),
                mf3[:, i * T : (i + 1) * T, 0:1],
                op=Alu.mult,
            )

        po129s = [None] * NI

        def pool_mms(i):
            xt = xts[i]
            pps = poolp.tile([1, D + 1], dt.float32, tag="p", name=f"pps_{i}")
            for t in range(T):
                nc.tensor.matmul(
                    pps[:, :],
                    P[:, 2 * (i * T + t) : 2 * (i * T + t) + 1],
                    xt[:, t * BL : t * BL + D + 1],
                    start=(t == 0),
                    stop=(t == T - 1),
                )
            # single PSUM reader: scalar copies [pooled | Z] to SBUF first
            po129 = vp.tile([1, D + 1], dt.float32, tag="po", name=f"po_{i}")
            nc.scalar.copy(po129[:, :], pps[:, :])
            po129s[i] = po129

        def finish(i):
            po129 = po129s[i]
            nc.vector.reciprocal(zinv[0:1, i : i + 1], po129[0:1, D : D + 1])
            nc.scalar.activation(
                outt[0:1, i * D : (i + 1) * D],
                po129[0:1, 0:D],
                Act.Copy,
                scale=zinv[0:1, i : i + 1],
            )

        # Software pipeline. Engines execute their instruction streams strictly
        # in order, so the emission order below is chosen so that every op's
        # cross-engine dependencies completed at least half a cycle earlier:
        #   DVE:    Pmult(i-1) | TT+L1-3(i) | STT(i+1) | L4-7(i) | recip(i-1)
        #   PE:     mcps(i+1) | mean(i+2) | vT(i+1) | pooled(i-1) | vb(i+1)
        #   Scalar: mcol(i+1) | meanT(i+2) | po(i-1) | exp(i) | vb/out copies
        for j in range(4):
            load_phase(j)
        mean_mms(0)
        mean_mms(1)
        chain_a(0)
        chain_vT(0)
        chain_stt(0)
        chain_vb(0)
        for i in range(NI):
            if i + 4 < NI:
                load_phase(i + 4)
            if i >= 1:
                pmult_phase(i - 1)
            if i + 1 < NI:
                chain_a(i + 1)
            if i + 2 < NI:
                mean_mms(i + 2)
            attn_dve(i)
            if i + 1 < NI:
                chain_vT(i + 1)
                chain_stt(i + 1)
            if i >= 1:
                pool_mms(i - 1)
            dve_tail(i)
            if i + 1 < NI:
                chain_vb(i + 1)
            if i >= 1:
                finish(i - 1)
        pmult_phase(NI - 1)
        pool_mms(NI - 1)
        finish(NI - 1)

        nc.sync.dma_start(out_d[:, :], outt[:, :])

    nc.compile()
    return nc


def _get_nc():
    if "nc" not in _CACHE:
        _CACHE["nc"] = _build()
    return _CACHE["nc"]


def _pack_inputs(x, mask):
    """Host-side layout prep: bf16 Xt-fold with ones/pad columns, mask fold."""
    import ml_dtypes

    bf16 = ml_dtypes.bfloat16
    B, N, d, H, W = x.shape
    M = B * N  # 128 items
    xr = np.asarray(x, dtype=np.float32).reshape(M, d, T, d)  # [item, d, t, p]
    xt = np.transpose(xr, (0, 3, 2, 1))  # [item, p, t, d]
    xtp = np.zeros((M, d, T, BL), dtype=bf16)
    xtp[:, :, :, 0:d] = xt.astype(bf16)
    xtp[:, :, :, d] = np.asarray(1.0, dtype=bf16)
    xtp = xtp.reshape(M, d, T * BL)

    mr = np.asarray(mask, dtype=np.float32).reshape(M, T, d)  # [item, t, p]
    mfo = np.transpose(mr, (0, 2, 1)).astype(bf16)  # [item, p, t]
    return xtp, mfo


def _make_in_maps(inputs):
    x, mask = inputs["x"], inputs["mask"]
    Wq, bq, Wk = inputs["Wq"], inputs["bq"], inputs["Wk"]
    xtp, mfo = _pack_inputs(x, mask)
    wqc = np.ascontiguousarray(Wq.astype(np.float32))
    wkc = np.ascontiguousarray(Wk.astype(np.float32))
    bq2 = np.ascontiguousarray(bq.reshape(D, 1).astype(np.float32))
    in_maps = []
    for c in range(NCORES):
        s = slice(c * NI, (c + 1) * NI)
        mfc = np.zeros((D, NI * T, 2), dtype=mfo.dtype)
        mfc[:, :, 0] = np.transpose(mfo[s], (1, 0, 2)).reshape(D, NI * T)
        mfc = np.ascontiguousarray(mfc.reshape(D, NI * T * 2))
        in_maps.append(
            {
                "x": np.ascontiguousarray(xtp[s]),
                "mf": mfc,
                "Wq": wqc,
                "Wk": wkc,
                "bq": bq2,
            }
        )
    return in_maps


def _gather(res, inputs):
    B, N, d = inputs["x"].shape[:3]
    parts = [
        np.asarray(res.results[c]["out"], dtype=np.float32).reshape(NI, d)
        for c in range(NCORES)
    ]
    return np.concatenate(parts, axis=0).reshape(B, N, d)


def kernel(x, mask, Wq, bq, Wk, bk):
    from concourse.bass_utils import run_bass_kernel_spmd

    inputs = {"x": x, "mask": mask, "Wq": Wq, "bq": bq, "Wk": Wk, "bk": bk}
    nc = _get_nc()
    in_maps = _make_in_maps(inputs)
    res = run_bass_kernel_spmd(nc, in_maps, core_ids=list(range(NCORES)))
    return _gather(res, inputs)
